# revision 1
# baseline (speedup 1.0000x reference)
# Trainium2 Bass kernel for nn_CustomConv2D_57200374448719:
#   data [32,128,64,64] f32 (NCHW) conv weights [256,128,3,3] (OIHW),
#   VALID, stride 1 -> out [32,256,62,62] f32.
#
# Strategy: data-parallel over batch across 8 NeuronCores (4 images per
# core), weights replicated. Per core, implicit GEMM with C_in=128 on the
# SBUF partition axis: for each image / C_out half (128) / group of 8
# output rows, accumulate 9 matmuls (one per 3x3 tap, K=128, N=rows*62)
# into one PSUM bank. The shifted conv windows are strided access
# patterns on the resident image tiles, so no im2col copy is ever
# materialized. Matmuls run in float32r (bit-identical fp32 in memory,
# reduced-precision multiply at full PE rate; measured error identical to
# the hardware fp32 path); accumulation is fp32 in PSUM.
#
# Startup-latency hiding: weights are loaded as two per-co-half chunks
# and each image as two halo'd row-halves, all on the sync-engine HWDGE
# ring, ordered so the first row-group's dependencies (first weight half
# + first image half) land as early as possible. Output stores go on the
# scalar-engine HWDGE ring (separate FIFO) per row-group so they stream
# out during compute.
import numpy as np

N_CORES = 8
B, CIN, H, W = 32, 128, 64, 64
COUT, KH, KW = 256, 3, 3
OH, OW = H - KH + 1, W - KW + 1  # 62, 62
BPC = B // N_CORES  # images per core
ROW_GROUPS = [(r0, min(8, OH - r0)) for r0 in range(0, OH, 8)]  # 7x8 + 1x6
# image row chunks (with conv halo): rows [0,10) serves row-group 0,
# [8,18) serves 1, [16,34) serves 2-3, [32,64) serves 4-7. The first two
# are small so the first matmuls' DMA dependencies land early.
CHUNKS = [(0, 10), (8, 10), (16, 18), (32, 32)]

_cache = {}


def build_nc(mm_dtype_name="float32r"):
    import concourse.bacc as bacc
    import concourse.mybir as mybir
    import concourse.tile as tile

    mm_dt = getattr(mybir.dt, mm_dtype_name)
    f32 = mybir.dt.float32

    nc = bacc.Bacc("TRN2", target_bir_lowering=False, debug=False, num_devices=N_CORES)
    data_in = nc.dram_tensor("data", [BPC, CIN, H, W], mm_dt, kind="ExternalInput").ap()
    # wt[ci, g*(9*128) + t*128 + co'] = weights[g*128+co', ci, ky, kx], t=ky*3+kx
    w_in = nc.dram_tensor("wt", [CIN, KH * KW * COUT], mm_dt, kind="ExternalInput").ap()
    out = nc.dram_tensor("out", [BPC, COUT, OH, OW], f32, kind="ExternalOutput").ap()
    WG = KH * KW * 128  # columns per co-half weight chunk

    with tile.TileContext(nc) as tc:
        with (
            tc.tile_pool(name="wpool", bufs=1) as wpool,
            tc.tile_pool(name="scr", bufs=1) as spool,
            tc.tile_pool(name="dpool", bufs=2) as dpool,
            tc.tile_pool(name="opool", bufs=6) as opool,
            tc.tile_pool(name="psum", bufs=8, space="PSUM") as ppool,
        ):
            # PE warm-up: the HAM clock gate holds the PE at 1.2 GHz until
            # ~3.4us of sustained activity, and the first ~14us here are
            # DMA-bound (preamble + weight/image loads). Run float32r dummy
            # matmuls on scratch data spanning that window so real matmuls
            # start at 2.4 GHz. (fp32 dummies don't work: they lower to
            # LOW_HIGH pairs, take 2-3x longer than budgeted, and the
            # sustained draw downclocks the whole stream.)
            if mm_dtype_name == "float32r":
                wscr = spool.tile([128, 512], f32)
                nc.gpsimd.memset(wscr[:], 0.0)
                wsr = wscr[:].bitcast(mm_dt)
            else:
                wscr = spool.tile([128, 512], mm_dt)
                nc.gpsimd.memset(wscr[:], 0.0)
                wsr = wscr[:]
            # the warm-up PSUM tile shares the main pool's slots (it is
            # long released by the time the 8th real group needs its bank)
            wps = ppool.tile([128, 512], f32, tag="ps")
            for _ in range(14):
                nc.tensor.matmul(wps[:], wsr[:, :128], wsr[:], start=True, stop=True)

            # weight chunks: (taps 0-2 of co-half 0) first — the smallest
            # prefix that lets matmuls begin — then the rest of half 0,
            # then half 1 (not needed until ~halfway through image 0).
            wt_g0a = wpool.tile([CIN, 3 * 128], mm_dt, tag="wt0a")
            wt_g0b = wpool.tile([CIN, 6 * 128], mm_dt, tag="wt0b")
            wt_g1 = wpool.tile([CIN, WG], mm_dt, tag="wt1")
            nc.sync.dma_start(wt_g0a[:], w_in[:, : 3 * 128])

            def wslice(g, t):
                if g == 1:
                    return wt_g1[:, t * 128 : (t + 1) * 128]
                if t < 3:
                    return wt_g0a[:, t * 128 : (t + 1) * 128]
                return wt_g0b[:, (t - 3) * 128 : (t - 2) * 128]

            dtiles = []
            for n in range(BPC):
                # +2 pad columns: the contiguous N=rows*64 matmul windows
                # read up to 2 elements past the last image row (garbage
                # output columns that are never copied out); fill them with
                # arbitrary real data to keep reads in-bounds and finite.
                chunks = []
                flat = data_in[n].rearrange("c h w -> c (h w)")
                for ci, (c0, crows) in enumerate(CHUNKS):
                    ct = dpool.tile([CIN, crows * W + 2], mm_dt, tag=f"d{ci}")
                    if (c0 + crows) * W + 2 <= H * W:
                        nc.sync.dma_start(
                            ct[:], flat[:, c0 * W : (c0 + crows) * W + 2]
                        )
                    else:
                        nc.sync.dma_start(
                            ct[:, : crows * W], flat[:, c0 * W : (c0 + crows) * W]
                        )
                        nc.sync.dma_start(ct[:, crows * W :], flat[:, :2])
                    chunks.append(ct)
                    if n == 0 and ci == 0:
                        nc.sync.dma_start(wt_g0b[:], w_in[:, 3 * 128 : WG])
                    if n == 0 and ci == len(CHUNKS) - 1:
                        nc.sync.dma_start(wt_g1[:], w_in[:, WG:])
                dtiles.append(chunks)

            def rhs_for(chunks, r0, rows, t):
                ci = next(
                    i
                    for i, (c0, crows) in enumerate(CHUNKS)
                    if r0 >= c0 and r0 + rows + KH - 1 <= c0 + crows
                )
                hr0 = r0 - CHUNKS[ci][0]
                ky, kx = divmod(t, KW)
                base = (hr0 + ky) * W + kx
                return chunks[ci][:, base : base + rows * W]

            def evacuate(n, g, r, r0, rows, ps):
                ot = opool.tile([128, 8 * OW], f32, tag="ot")
                src = ps[:].rearrange("p (r w) -> p r w", w=W)[:, :, :OW]
                dst = ot[:, : rows * OW].rearrange("p (r w) -> p r w", w=OW)
                if r % 2 == 0:
                    nc.vector.tensor_copy(dst, src)
                else:
                    nc.scalar.copy(dst, src)
                nc.scalar.dma_start(
                    out[n].rearrange("c h w -> c (h w)")[
                        g * 128 : (g + 1) * 128, r0 * OW : (r0 + rows) * OW
                    ],
                    ot[:, : rows * OW],
                )

            for n in range(BPC):
                chunks = dtiles[n]
                for g in range(COUT // 128):
                    for r, (r0, rows) in enumerate(ROW_GROUPS):
                        ps = ppool.tile([128, rows * W], f32, tag="ps")
                        for t in range(KH * KW):
                            nc.tensor.matmul(
                                ps[:],
                                wslice(g, t),
                                rhs_for(chunks, r0, rows, t),
                                start=(t == 0),
                                stop=(t == KH * KW - 1),
                            )
                        evacuate(n, g, r, r0, rows, ps)
    nc.compile()
    return nc


def _get_nc(mm_dtype_name="float32r"):
    if mm_dtype_name not in _cache:
        _cache[mm_dtype_name] = build_nc(mm_dtype_name)
    return _cache[mm_dtype_name]


def _get_runner(mm_dtype_name="float32r"):
    """Build the 8-core PJRT executable once and cache it: repeat kernel()
    calls then skip bass2jax's per-call jit re-trace (~6s each)."""
    key = ("runner", mm_dtype_name)
    if key in _cache:
        return _cache[key]

    import jax
    import jax.core
    from jax.experimental.shard_map import shard_map
    from jax.sharding import Mesh, PartitionSpec

    import concourse.mybir as mybir
    from concourse import bass2jax

    nc = _get_nc(mm_dtype_name)
    bass2jax.install_neuronx_cc_hook()

    partition_name = nc.partition_id_tensor.name if nc.partition_id_tensor else None
    in_names, out_names, out_avals = [], [], []
    for alloc in nc.m.functions[0].allocations:
        if not isinstance(alloc, mybir.MemoryLocationSet):
            continue
        name = alloc.memorylocations[0].name
        if alloc.kind == "ExternalInput":
            if name != partition_name:
                in_names.append(name)
        elif alloc.kind == "ExternalOutput":
            out_names.append(name)
            out_avals.append(
                jax.core.ShapedArray(
                    tuple(alloc.tensor_shape), mybir.dt.np(alloc.dtype)
                )
            )
    n_params, n_outs = len(in_names), len(out_names)
    all_names = in_names + out_names
    if partition_name is not None:
        all_names = all_names + [partition_name]
    donate = tuple(range(n_params, n_params + n_outs))

    def _body(*args):
        operands = list(args)
        if partition_name is not None:
            operands.append(bass2jax.partition_id_tensor())
        outs = bass2jax._bass_exec_p.bind(
            *operands,
            out_avals=tuple(out_avals),
            in_names=tuple(all_names),
            out_names=tuple(out_names),
            lowering_input_output_aliases=(),
            sim_require_finite=True,
            sim_require_nnan=True,
            nc=nc,
        )
        return tuple(outs)

    devices = jax.devices()[:N_CORES]
    mesh = Mesh(np.asarray(devices), ("core",))
    sharded = jax.jit(
        shard_map(
            _body,
            mesh=mesh,
            in_specs=(PartitionSpec("core"),) * (n_params + n_outs),
            out_specs=(PartitionSpec("core"),) * n_outs,
            check_rep=False,
        ),
        donate_argnums=donate,
        keep_unused=True,
    )
    runner = (in_names, out_names, out_avals, sharded)
    _cache[key] = runner
    return runner


def _np_in_dtype(mm_dtype_name):
    if mm_dtype_name == "bfloat16":
        import ml_dtypes

        return ml_dtypes.bfloat16
    if mm_dtype_name == "float16":
        return np.float16
    return np.float32


def _prep_weights(weights, np_dt):
    # [co, ci, ky, kx] -> [ci][t=ky*3+kx][g][co'] -> [ci][g][t][co'] flat
    w4 = np.asarray(weights, dtype=np.float32).transpose(1, 2, 3, 0)  # ci,ky,kx,co
    w4 = w4.reshape(CIN, KH * KW, COUT // 128, 128).transpose(0, 2, 1, 3)
    return np.ascontiguousarray(w4, dtype=np_dt).reshape(CIN, KH * KW * COUT)


def kernel(data: np.ndarray, weights: np.ndarray, _dtype="float32r") -> np.ndarray:
    np_dt = _np_in_dtype(_dtype)
    data = np.ascontiguousarray(np.asarray(data), dtype=np_dt)
    wt = _prep_weights(weights, np_dt)

    in_names, out_names, out_avals, sharded = _get_runner(_dtype)
    # shard_map splits axis 0 across the 8 cores: the global batch-sharded
    # arrays are exactly the full input (batch 32 -> 4 per core) and the
    # per-core-replicated weights tiled 8x on axis 0.
    globals_ = {
        "data": data.reshape(N_CORES * BPC, CIN, H, W),
        "wt": np.tile(wt, (N_CORES, 1)),
    }
    args = [globals_[n] for n in in_names] + [
        np.zeros((N_CORES * av.shape[0], *av.shape[1:]), av.dtype)
        for av in out_avals
    ]
    outs = sharded(*args)
    return np.asarray(outs[out_names.index("out")])

